# revision 1
# baseline (speedup 1.0000x reference)
"""Trainium2 Bass kernel for nn_CViTFlow (cross-attention ViT flow block).

Math (per the module):
  two token streams x1,x2 [B,T,256] viewed as [B,256,48,48] images.
  6 branches (q1,k1,v1,q2,k2,v2): depthwise3x3 -> BN(eval) -> 1x1 conv -> Linear.
  o1 = softmax(-(q1 k2^T / 16)) v2 + q1 ;  o2 = softmax(-(q2 k1^T / 16)) v1 + q2
  both reshaped [B,H,T,DH] -> [B,T,256] with a plain (head-major) reshape.

Kernel strategy:
  * Host folds BN + 1x1conv + Linear into one 256x256 matrix W and bias c per
    branch, then folds the depthwise 3x3 into 9 "tap" matrices
    Wtap[o,c] = W[o,c]*dw[c,di,dj], so a whole branch becomes 9 shifted
    matmuls accumulated in PSUM (all compute lands on the TensorEngine).
  * Host pre-transposes/pads images to channel-major [256, 50, 50] (zero pad)
    so tap shifts are plain strided access patterns.
  * 8 cores = (map m) x (batch b) x (head-quad g): each core computes one
    output map's 4 heads for one batch element. No collectives.
  * On device: branch matmuls produce qT/kT/vT [128=4*32, T]; v is
    PE-transposed to [t,d] tiles augmented with a ones column (so the AV
    matmul also produces the softmax denominator).
  * Scores: the 4 heads' K=32 matmuls run CONCURRENTLY in 4 PE row strips
    (tile_position=(32h,0)) reading kT/qT directly at partitions 32h..32h+32,
    each draining into its OWN PSUM bank (concurrent drains to distinct banks
    are safe; same-bank same-partition drains crash).  Scores live in two
    persistent 2-bank tiles (scA: heads 0,1 / scB: heads 2,3); within a
    head's bank, j-parity picks the 256-col half.
  * exp on ScalarE, once per j-PAIR as two CONTIGUOUS activations (all of
    scA, then all of scB) so the dep tracker's bounding boxes are exact and
    consecutive pairs' exps chain back-to-back (~93% ScalarE busy); AV
    accumulates over t in PSUM (2 col-strip-concurrent pairs); finalize
    divides by the denominator (DVE reciprocal + PE broadcast) and adds the
    q residual.
"""

import numpy as np

B = 2
T = 2304
DIM = 256
HEADS = 8
DH = 32
HW = 48
EPS = 1e-5
P = 128
N_CORES = 8

# t-tiles for the branch phase: row-aligned in the 48x48 image (10/8 rows)
T_TILES = [(0, 480, 0, 10), (480, 480, 10, 10), (960, 480, 20, 10),
           (1440, 480, 30, 10), (1920, 384, 40, 8)]
# image-row bands for the input DMAs: tile k's taps read rows r0..r0+nr+1,
# so band boundaries at row cuts let each branch tile start as soon as the
# bands covering its rows have landed
DMA_BANDS = [(0, 12), (12, 22), (22, 32), (32, 42), (42, 50)]
NL = 256
N_LT = T // NL  # 9
N_TCH = T // P  # 18 t-chunks of 128 for scores/AV

_PROGRAM = None  # cached Bass program
_last_in_maps = None  # stashed per-core input maps (for external profiling runs)


def _build_program(debug=False):
    """Build the SPMD Bass/Tile program (identical for all 8 cores)."""
    import sys
    import types
    from contextlib import ExitStack

    # the rust AP-lowering path does `import log` when it encounters a
    # not-yet-bound virtual tensor (used for the scq exp-read alias below);
    # the module doesn't exist in this image — stub it
    if "log" not in sys.modules:
        _log = types.ModuleType("log")
        for _fn in ("debug", "info", "warning", "warn", "error", "critical",
                    "exception"):
            setattr(_log, _fn, lambda *a, **k: None)
        sys.modules["log"] = _log

    import concourse.bacc as bacc
    import concourse.mybir as mybir
    import concourse.tile as tile
    from concourse.masks import make_identity
    from concourse.tile_rust import add_dep_helper

    f32 = mybir.dt.float32
    f32r = mybir.dt.float32r
    bf16 = mybir.dt.bfloat16
    AF = mybir.ActivationFunctionType
    OP = mybir.AluOpType

    # Bacc (not raw Bass): its compile() runs move_matmul_waits_to_ldweights +
    # generate_event_semaphores, without which walrus rejects multi-wait matmuls
    nc = bacc.Bacc(None, target_bir_lowering=False, debug=False)

    # DRAM I/O (per core).  pad_a feeds the q branch, pad_b feeds k and v.
    # Matmul operands are bf16 (fp32 matmuls cost 2 PE passes via LOW_HIGH
    # mode); accumulation stays fp32 in PSUM.
    pad_a = nc.declare_dram_parameter("pad_a", [2, P, 2500], bf16, isOutput=False)
    pad_b = nc.declare_dram_parameter("pad_b", [2, P, 2500], bf16, isOutput=False)
    wq = nc.declare_dram_parameter("wq", [2, P, 9 * P], bf16, isOutput=False)
    wk = nc.declare_dram_parameter("wk", [2, P, 9 * P], bf16, isOutput=False)
    wv = nc.declare_dram_parameter("wv", [2, P, 9 * P], bf16, isOutput=False)
    bias_d = nc.declare_dram_parameter("bias", [3, P, 1], f32, isOutput=False)
    out_d = nc.declare_dram_parameter("out", [P, T], f32, isOutput=True)

    with tile.TileContext(nc) as tc, ExitStack() as ctx:
        const = ctx.enter_context(tc.tile_pool(name="const", bufs=1))
        sb = ctx.enter_context(tc.tile_pool(name="sb", bufs=1))
        fin = ctx.enter_context(tc.tile_pool(name="fin", bufs=2))

        identity = const.tile([P, P], bf16)
        make_identity(nc, identity)
        ones32 = const.tile([1, 32], f32r)
        # memset can't target f32r; 1.0 has identical f32/f32r bits
        nc.vector.memset(ones32[:].bitcast(f32), 1.0)

        # ---- input DMAs (k weights + image B first: they gate phase A).
        # Images stream in row bands so the first branch tile only waits for
        # its own rows, not the whole 640KB image.
        wk_sb = sb.tile([P, 2 * 9 * P], bf16, tag="wk")
        pb_sb = sb.tile([P, 2 * 2500], bf16, tag="pb")
        wv_sb = sb.tile([P, 2 * 9 * P], bf16, tag="wv")
        wq_sb = sb.tile([P, 2 * 9 * P], bf16, tag="wq")
        pa_sb = sb.tile([P, 2 * 2500], bf16, tag="pa")
        bias_sb = sb.tile([P, 3], f32, tag="bias")
        def _img_band(dst, src, kc, r0, r1):
            nc.sync.dma_start(dst[:, kc * 2500 + r0 * 50:kc * 2500 + r1 * 50],
                              src[kc][:, r0 * 50:r1 * 50])

        # k weights + first image-B band gate the first matmul; v/q weights
        # next (their branches start while later bands stream); remaining
        # image bands interleaved B-then-A
        for kc in range(2):
            nc.sync.dma_start(wk_sb[:, kc * 1152:(kc + 1) * 1152], wk[kc])
        for kc in range(2):
            _img_band(pb_sb, pad_b, kc, *DMA_BANDS[0])
        for kc in range(2):
            nc.sync.dma_start(wv_sb[:, kc * 1152:(kc + 1) * 1152], wv[kc])
        for kc in range(2):
            nc.sync.dma_start(wq_sb[:, kc * 1152:(kc + 1) * 1152], wq[kc])
        for r in range(3):
            nc.sync.dma_start(bias_sb[:, r:r + 1], bias_d[r])
        for i, (r0, r1) in enumerate(DMA_BANDS):
            if i > 0:
                for kc in range(2):
                    _img_band(pb_sb, pad_b, kc, r0, r1)
            for kc in range(2):
                _img_band(pa_sb, pad_a, kc, r0, r1)

        qT = sb.tile([P, T], bf16, tag="qT")
        qTf = sb.tile([P, T], f32, tag="qTf")   # fp32 copy for the residual
        kT = sb.tile([P, T], bf16, tag="kT")
        vT = sb.tile([P, T], bf16, tag="vT")
        # per t-chunk, per head: 64 cols = [v(32) | ones(1) | zeros(31)] so AV
        # pairs col-tile at (0,0)/(0,64) and the ones column carries the
        # softmax denominator through the same matmul
        vaug = sb.tile([P, N_TCH * 256], bf16, tag="vaug")
        outbuf = sb.tile([P, T], f32, tag="outbuf")

        # One PSUM pool for the whole kernel (a pool transition emits barrier
        # boundaries -> a PE idle gap that drops the HAM clock to 1.2 GHz for
        # the rest of the kernel).  Bank budget (8 banks of 2KB):
        #   scq   4 banks  persistent score region (head h -> bank h)
        #   avout 2 banks  AV accumulators; branch/transpose tiles borrow it
        #   bc    2 banks  denominator broadcast tiles
        psumB = ctx.enter_context(tc.tile_pool(name="psum", bufs=2, space="PSUM"))
        ep = ctx.enter_context(tc.tile_pool(name="ep", bufs=2))

        # Score regions: two persistent 2-bank tiles.  scA holds heads 0,1
        # (bank per head), scB heads 2,3; within a head's bank, j-parity
        # picks the 256-col half.  Why two tiles + pair-wise exp: the dep
        # tracker coarsens strided APs to bounding boxes, so an exp that
        # reads one parity strided across banks would falsely depend on the
        # NEXT tile's score matmuls and serialize the pipeline.  Instead exp
        # runs once per j-PAIR as two CONTIGUOUS reads (all of scA, then all
        # of scB) whose tracked ranges are exact: scores of pair p+1 for
        # heads 0,1 only WAR exp-A(p), so they run while exp-B(p) is still
        # on the ScalarE and the two exps of consecutive pairs chain
        # back-to-back.
        scA = psumB.tile([P, 2 * 512], f32, tag="scA", bufs=1, name="scA")
        scB = psumB.tile([P, 2 * 512], f32, tag="scB", bufs=1, name="scB")

        # ================= Phase A: branch matmuls =================
        def branch(w_sb, img_sb, dest, role, dest2=None):
            for (t0, nt, r0, nr) in T_TILES:
                ps = psumB.tile([P, nt], f32, tag="avout", bufs=2,
                                name=f"br_{role}_{t0}")
                mm = 0
                for kc in range(2):
                    pv = img_sb[:, kc * 2500:(kc + 1) * 2500].rearrange(
                        "p (r c) -> p r c", c=50)
                    wv_ = w_sb[:, kc * 1152:(kc + 1) * 1152]
                    for di in range(3):
                        for dj in range(3):
                            tap = di * 3 + dj
                            rhs = pv[:, r0 + di:r0 + di + nr, dj:dj + 48]
                            nc.tensor.matmul(
                                ps[:], wv_[:, tap * P:(tap + 1) * P], rhs,
                                start=(mm == 0), stop=(mm == 17))
                            mm += 1
                # bias add, PSUM -> SBUF (bf16 for matmul operands)
                nc.vector.tensor_scalar_add(dest[:, t0:t0 + nt], ps[:],
                                            bias_sb[:, role:role + 1])
                if dest2 is not None:
                    nc.vector.tensor_scalar_add(dest2[:, t0:t0 + nt], ps[:],
                                                bias_sb[:, role:role + 1])

        branch(wk_sb, pb_sb, kT, 1)
        branch(wv_sb, pb_sb, vT, 2)

        # v: transpose to [t, d] tiles, 64 cols per head.  These run BEFORE
        # the q branch: transpose-mode matmuls don't count as PE-busy for the
        # HAM clock gate, so putting them last would idle-window the PE into
        # its 1.2 GHz throttle right at attention start. The dense q-branch
        # matmuls after them re-warm and carry 2.4 GHz into attention.
        nc.vector.memset(vaug[:], 0.0)
        for j in range(N_TCH):
            tp = psumB.tile([P, 512], bf16, tag="avout", bufs=2, name=f"tp_{j}")
            nc.tensor.transpose(tp[:, 0:P], vT[:, j * P:(j + 1) * P], identity[:])
            dst = vaug[:, j * 256:(j + 1) * 256].rearrange(
                "p (h c) -> p h c", c=64)[:, :, 0:32]
            src = tp[:, 0:P].rearrange("p (h c) -> p h c", c=32)
            nc.vector.tensor_copy(dst, src)
        ones_cols = vaug.rearrange("p (j h c) -> p j h c", h=4, c=64)[:, :, :, 32:33]
        nc.vector.memset(ones_cols, 1.0)

        branch(wq_sb, pa_sb, qT, 0, dest2=qTf)

        # ================= Phase B: attention =================
        # Software-pipelined so the in-order PE stream never sits behind an
        # exp wait: scores(t+1) issue before AV(t); finalize(l-1) is injected
        # mid-way through l's t-loop (its bc matmuls wait on DVE reciprocals).

        def scores(j):
            """4 heads concurrent in 4 row strips -> 4 distinct PSUM banks."""
            li, par = j // N_TCH, (j % 2) * NL
            l0 = li * NL
            last = None
            for h in range(4):
                dst = scA if h < 2 else scB
                last = nc.tensor.matmul(
                    dst[:, (h % 2) * 512 + par:(h % 2) * 512 + par + NL],
                    kT[32 * h:32 * h + 32, (j % N_TCH) * P:(j % N_TCH + 1) * P],
                    qT[32 * h:32 * h + 32, l0:l0 + NL],
                    start=True, stop=True, tile_position=(32 * h, 0))
            return last

        def finalize_recips(li, outp):
            # issue right after the l-tile's AV accumulation completes: the
            # reciprocals then overlap the next l-tile's attention instead of
            # head-of-line-blocking the PE at the bc MMs
            recips = []
            for h in range(4):
                cp, pb_ = 256 * (h // 2), 64 * (h % 2)
                recip = fin.tile([1, NL], mybir.dt.float32r, tag="recip",
                                 bufs=8, name=f"recip_{li}_{h}")
                with nc.allow_low_precision(reason="f32r recip: 2^-19 rel "
                                            "is ample for softmax denom"):
                    nc.vector.reciprocal(recip[:],
                                         outp[pb_ + 32:pb_ + 33, cp:cp + NL])
                recips.append(recip)
            return recips

        def finalize_head(li, outp, recips, h):
            l0 = li * NL
            cp, pb_ = 256 * (h // 2), 64 * (h % 2)
            # float32r: single PE pass (plain fp32 lowers to 2 LOW_HIGH
            # passes); ~2^-19 relative precision is plenty for 1/denom
            bc_ps = psumB.tile([32, NL], f32, tag="bc", bufs=2,
                               name=f"bc_{li}_{h}")
            nc.tensor.matmul(bc_ps[:], ones32[:], recips[h][:],
                             start=True, stop=True)
            bc_sb = fin.tile([32, NL], f32, tag="bcsb", bufs=4)
            nc.vector.tensor_copy(bc_sb[:], bc_ps[:])
            # av * (1/denom): PSUM+SBUF inputs may differ in base
            # partition (only SB+SB pairs must match), out lands at 32h
            nc.vector.tensor_tensor(outbuf[h * 32:(h + 1) * 32, l0:l0 + NL],
                                    outp[pb_:pb_ + 32, cp:cp + NL],
                                    bc_sb[:], op=OP.mult)
            # in-place residual: both SB inputs at base partition 32h
            nc.vector.tensor_tensor(outbuf[h * 32:(h + 1) * 32, l0:l0 + NL],
                                    outbuf[h * 32:(h + 1) * 32, l0:l0 + NL],
                                    qTf[h * 32:(h + 1) * 32, l0:l0 + NL],
                                    op=OP.add)
            if h == 3:
                # stream this l-slice out now instead of one big tail DMA
                nc.sync.dma_start(out_d[:, l0:l0 + NL], outbuf[:, l0:l0 + NL])

        NPAIR = N_TCH // 2  # 9 j-pairs per l-tile

        def do_exp(p):
            """Pair p: two contiguous ACTIVATEs (all of scA, all of scB)."""
            etA = ep.tile([P, 1024], bf16, tag="eA", name=f"etA_{p}")
            etB = ep.tile([P, 1024], bf16, tag="eB", name=f"etB_{p}")
            nc.scalar.activation(etA[:], scA[:], AF.Exp, scale=-0.0625)
            nc.scalar.activation(etB[:], scB[:], AF.Exp, scale=-0.0625)
            return (etA, etB)

        def av_tile(j, ets, outp):
            jl, par = j % N_TCH, (j % 2) * NL
            for h in range(4):
                cp, sub = 256 * (h // 2), h % 2
                src = ets[0] if h < 2 else ets[1]
                nc.tensor.matmul(
                    outp[64 * sub:64 * sub + 64, cp:cp + NL],
                    vaug[:, jl * 256 + 64 * h:jl * 256 + 64 * h + 64],
                    src[:, (h % 2) * 512 + par:(h % 2) * 512 + par + NL],
                    start=(jl == 0), stop=(jl == N_TCH - 1),
                    tile_position=(0, 64 * sub),
                    skip_group_check=True)

        prev = None  # (li, outp, recips) awaiting finalize
        scores(0)
        scores(1)
        ets_next = do_exp(0)
        for li in range(N_LT):
            # one bank: pair cp at cols 256*cp, sub s at partitions 64s
            outp = psumB.tile([P, 2 * NL], f32, tag="avout", bufs=2,
                              name=f"avout_{li}")
            for pp in range(NPAIR):
                p = li * NPAIR + pp
                j0 = 2 * p
                ets = ets_next
                if debug and p == 0:
                    dbg_sc = nc.declare_dram_parameter(
                        "dbg_sc0", [P, 2048], f32, isOutput=True)
                    sc_sb = sb.tile([P, 2048], f32, tag="dbgsc")
                    nc.vector.tensor_copy(sc_sb[:, 0:1024], scA[:])
                    nc.vector.tensor_copy(sc_sb[:, 1024:2048], scB[:])
                    nc.sync.dma_start(dbg_sc[:], sc_sb[:])
                    dbg_et = nc.declare_dram_parameter(
                        "dbg_et0", [P, 2048], bf16, isOutput=True)
                    nc.sync.dma_start(dbg_et[:, 0:1024], ets[0][:])
                    nc.sync.dma_start(dbg_et[:, 1024:2048], ets[1][:])
                if p + 1 < N_LT * NPAIR:
                    scores(j0 + 2)
                    scores(j0 + 3)
                    ets_next = do_exp(p + 1)
                av_tile(j0, ets, outp)
                av_tile(j0 + 1, ets, outp)
                # spread the finalize of l-1 over the middle of this loop
                # (one head per pair, so each PE insertion is small)
                if prev is not None and pp in (3, 4, 5, 6):
                    finalize_head(*prev, pp - 3)
                    if pp == 6:
                        prev = None
            prev = (li, outp, finalize_recips(li, outp))
        for h in range(4):
            finalize_head(*prev, h)

        if debug:
            for nm, t in [("dbg_qT", qT), ("dbg_kT", kT), ("dbg_vT", vT),
                          ("dbg_vaug", vaug), ("dbg_qTf", qTf)]:
                dd = nc.declare_dram_parameter(nm, list(t.shape), t.dtype,
                                               isOutput=True)
                nc.sync.dma_start(dd[:], t[:])

    nc.compile()
    return nc


def _fold_weights(dw_w, bn_gamma, bn_beta, bn_mean, bn_var, pw_w, pw_b, lin_w):
    """Fold BN + pointwise conv + linear (+ depthwise taps) per branch.

    Returns Wtap [6, 9, 256, 256] (float32) and bias c [6, 256]."""
    dw = dw_w.astype(np.float64)
    g = bn_gamma.astype(np.float64)
    b = bn_beta.astype(np.float64)
    m = bn_mean.astype(np.float64)
    v = bn_var.astype(np.float64)
    pw = pw_w.astype(np.float64)
    pb = pw_b.astype(np.float64)
    lw = lin_w.astype(np.float64)

    scale = g / np.sqrt(v + EPS)                      # [6, 256]
    shift = b - m * scale                             # [6, 256]
    M = np.einsum("noc,ncd->nod", lw, pw)             # lin @ pw  [6, 256, 256]
    W = M * scale[:, None, :]                         # [6, 256(o), 256(c)]
    c = np.einsum("noc,nc->no", M, shift) + np.einsum("noc,nc->no", lw, pb)
    # taps: Wtap[n, di*3+dj, o, c] = W[n, o, c] * dw[n, c, di, dj]
    Wtap = W[:, None, :, :] * dw.transpose(0, 2, 3, 1).reshape(6, 9, 1, 256)
    return Wtap.astype(np.float32), c.astype(np.float32)


def _bf16(a):
    import ml_dtypes
    return a.astype(ml_dtypes.bfloat16)


def _pad_images(x):
    """x [B, T, 256] -> per batch channel-major zero-padded bf16 [2,128,2500]."""
    out = np.zeros((B, 2, P, 50, 50), dtype=np.float32)
    img = np.ascontiguousarray(x.transpose(0, 2, 1)).reshape(B, DIM, HW, HW)
    out[:, :, :, 1:49, 1:49] = img.reshape(B, 2, P, HW, HW)
    return _bf16(out.reshape(B, 2, P, 2500))


def _wtap_lhsT(Wtap, branch, g):
    """Pack lhsT layout [2, 128, 9*128] for a branch restricted to quad g."""
    rows = slice(g * P, (g + 1) * P)
    out = np.empty((2, P, 9 * P), dtype=np.float32)
    for kc in range(2):
        for tap in range(9):
            blk = Wtap[branch, tap][rows, kc * P:(kc + 1) * P]  # [128 o, 128 c]
            out[kc, :, tap * P:(tap + 1) * P] = blk.T
    return _bf16(out)


def kernel(x1, x2, dw_w, bn_gamma, bn_beta, bn_mean, bn_var, pw_w, pw_b, lin_w,
           h1=HW, w1=HW, h2=HW, w2=HW):
    global _PROGRAM
    from concourse.bass_utils import run_bass_kernel_spmd

    x1 = np.asarray(x1, dtype=np.float32)
    x2 = np.asarray(x2, dtype=np.float32)

    Wtap, c = _fold_weights(np.asarray(dw_w), np.asarray(bn_gamma),
                            np.asarray(bn_beta), np.asarray(bn_mean),
                            np.asarray(bn_var), np.asarray(pw_w),
                            np.asarray(pw_b), np.asarray(lin_w))
    pad1 = _pad_images(x1)   # [B, 2, 128, 2500]
    pad2 = _pad_images(x2)

    if _PROGRAM is None:
        _PROGRAM = _build_program()
    nc = _PROGRAM

    # core layout: core = m*4 + b*2 + g
    # map m=0: o1 = att(q=br0(x1), k=br4(x2), v=br5(x2)) + q1
    # map m=1: o2 = att(q=br3(x2), k=br1(x1), v=br2(x1)) + q2
    in_maps = []
    for m in range(2):
        qbr, kbr, vbr = (0, 4, 5) if m == 0 else (3, 1, 2)
        pa, pb_ = (pad1, pad2) if m == 0 else (pad2, pad1)
        for b in range(2):
            for g in range(2):
                bias = np.stack([c[qbr, g * P:(g + 1) * P],
                                 c[kbr, g * P:(g + 1) * P],
                                 c[vbr, g * P:(g + 1) * P]])[:, :, None]
                in_maps.append({
                    "pad_a": np.ascontiguousarray(pa[b]),
                    "pad_b": np.ascontiguousarray(pb_[b]),
                    "wq": _wtap_lhsT(Wtap, qbr, g),
                    "wk": _wtap_lhsT(Wtap, kbr, g),
                    "wv": _wtap_lhsT(Wtap, vbr, g),
                    "bias": np.ascontiguousarray(bias),
                })

    global _last_in_maps
    _last_in_maps = in_maps
    res = run_bass_kernel_spmd(nc, in_maps, list(range(N_CORES)))

    o = np.empty((2, 2, HEADS, T, DH), dtype=np.float32)
    for m in range(2):
        for b in range(2):
            for g in range(2):
                core = m * 4 + b * 2 + g
                blk = res.results[core]["out"].reshape(4, DH, T)
                o[m, b, 4 * g:4 * g + 4] = blk.transpose(0, 2, 1)
    o1 = o[0].reshape(B, T, HEADS * DH)
    o2 = o[1].reshape(B, T, HEADS * DH)
    return o1, o2



# revision 11
# speedup vs baseline: 1.3959x; 1.3959x over previous
"""Trainium2 Bass kernel for nn_CViTFlow (cross-attention ViT flow block).

Math (per the module):
  two token streams x1,x2 [B,T,256] viewed as [B,256,48,48] images.
  6 branches (q1,k1,v1,q2,k2,v2): depthwise3x3 -> BN(eval) -> 1x1 conv -> Linear.
  o1 = softmax(-(q1 k2^T / 16)) v2 + q1 ;  o2 = softmax(-(q2 k1^T / 16)) v1 + q2
  both reshaped [B,H,T,DH] -> [B,T,256] with a plain (head-major) reshape.

Kernel strategy (v2 — single fused software pipeline):
  * Host folds BN + 1x1conv + Linear into one 256x256 matrix W and bias c per
    branch, then folds the depthwise 3x3 into 9 "tap" matrices, so a branch is
    9 shifted matmuls accumulated in PSUM.  8 cores = (map, batch, head-quad);
    no collectives.  (Same host-side prep as v1.)
  * The whole kernel is ONE pipeline paced by the ScalarE exp stream (the hard
    floor: 162 ACTIVATEs x [128,1024] ~= 216us).  No serial branch phase:
    - upfront: k-branch tiles interleaved with l-tile-0 scores+exp as kT
      chunks become available (exp starts ~12us in),
    - the main j-loop (one step per (l-tile, t-chunk)) carries v-branch,
      transposes, and q-branch tiles 2-4 as PE filler items, plus deferred AV.
  * Scores: ONE matmul per j (K=128, N=1024) using a block-diagonal expanded
    rhs qTx [128, 4*256] (head h's q strip in rows 32h, zeros elsewhere); the
    zero rows select each head's 32 dims out of the full-128 contraction.
    1 LDWEIGHTS (vs 4 strip loads) and a clean 2-bank PSUM tile per j.
  * Score tiles double-buffered (2 x 2 banks) -> exp(j) never WARs scores(j+1)
    and the exp chain runs back-to-back on ScalarE.
  * AV: per (j, head) matmul with M=33 weights [v(32) | ones] (LDWEIGHTS cost
    scales with columns: 33 not 64); ones column accumulates the softmax
    denominator.  AV is DEFERRED (emission gated on its vaug chunk having been
    emitted -> deadlock-free by construction) and catches up 2/step.
  * Finalize per l-tile: 4 denominator row-copies -> ONE [4,256] f32r
    reciprocal -> ONE mask4 broadcast matmul (K=4 -> all 128 rows) -> 4 mult +
    4 residual-add, then the output slice DMAs out.  Spread one substep per
    loop step.
  * PSUM (8 banks): sc 2x2 + avout 2x1 + scratch(branch/transpose/bc) 2x1.
"""

import numpy as np

B = 2
T = 2304
DIM = 256
HEADS = 8
DH = 32
HW = 48
EPS = 1e-5
P = 128
N_CORES = 8

# t-tiles for the branch phase: row-aligned in the 48x48 image (10/8 rows)
T_TILES = [(0, 480, 0, 10), (480, 480, 10, 10), (960, 480, 20, 10),
           (1440, 480, 30, 10), (1920, 384, 40, 8)]
# image-row bands for the input DMAs: tile k's taps read rows r0..r0+nr+1,
# so band boundaries at row cuts let each branch tile start as soon as the
# bands covering its rows have landed
DMA_BANDS = [(0, 12), (12, 22), (22, 32), (32, 42), (42, 50)]
NL = 256
N_LT = T // NL  # 9
N_TCH = T // P  # 18 t-chunks of 128 for scores/AV
NJ = N_LT * N_TCH  # 162 pipeline steps
# chunks fully covered after branch t-tile i has drained
TILE_CHUNKS = [(0, 3), (3, 7), (7, 11), (11, 15), (15, 18)]
ET_BUFS = 36  # et ring: AV may lag exp by up to ET_BUFS-2 steps

_PROGRAM = None  # cached Bass program
_last_in_maps = None  # stashed per-core input maps (for external profiling runs)


def _build_program(debug=False):
    """Build the SPMD Bass/Tile program (identical for all 8 cores)."""
    import sys
    import types
    from contextlib import ExitStack

    # the rust AP-lowering path does `import log` when it encounters a
    # not-yet-bound virtual tensor; the module doesn't exist in this image
    if "log" not in sys.modules:
        _log = types.ModuleType("log")
        for _fn in ("debug", "info", "warning", "warn", "error", "critical",
                    "exception"):
            setattr(_log, _fn, lambda *a, **k: None)
        sys.modules["log"] = _log

    import concourse.bacc as bacc
    import concourse.mybir as mybir
    import concourse.tile as tile
    from concourse.masks import make_identity

    f32 = mybir.dt.float32
    f32r = mybir.dt.float32r
    bf16 = mybir.dt.bfloat16
    AF = mybir.ActivationFunctionType
    OP = mybir.AluOpType

    # Bacc (not raw Bass): its compile() runs move_matmul_waits_to_ldweights +
    # generate_event_semaphores, without which walrus rejects multi-wait matmuls
    nc = bacc.Bacc(None, target_bir_lowering=False, debug=False)

    pad_a = nc.declare_dram_parameter("pad_a", [2, P, 2500], bf16, isOutput=False)
    pad_b = nc.declare_dram_parameter("pad_b", [2, P, 2500], bf16, isOutput=False)
    wq = nc.declare_dram_parameter("wq", [2, P, 9 * P], bf16, isOutput=False)
    wk = nc.declare_dram_parameter("wk", [2, P, 9 * P], bf16, isOutput=False)
    wv = nc.declare_dram_parameter("wv", [2, P, 9 * P], bf16, isOutput=False)
    bias_d = nc.declare_dram_parameter("bias", [3, P, 1], f32, isOutput=False)
    out_d = nc.declare_dram_parameter("out", [P, T], f32, isOutput=True)

    with tile.TileContext(nc) as tc, ExitStack() as ctx:
        const = ctx.enter_context(tc.tile_pool(name="const", bufs=1))
        sb = ctx.enter_context(tc.tile_pool(name="sb", bufs=1))
        fin = ctx.enter_context(tc.tile_pool(name="fin", bufs=2))
        ep = ctx.enter_context(tc.tile_pool(name="ep", bufs=2))
        psumB = ctx.enter_context(tc.tile_pool(name="psum", bufs=2, space="PSUM"))

        identity = const.tile([P, P], bf16)
        make_identity(nc, identity)
        # mask4[k, m] = 1 iff m//32 == k : K=4 broadcast matmul takes the 4
        # per-head reciprocal rows to all 128 output partitions in one shot.
        # Built on host (DVE memsets can't target partition bases 1..3).
        mask4_d = nc.declare_dram_parameter("mask4", [4, P], f32, isOutput=False)
        mask4f = const.tile([4, P], f32)
        nc.sync.dma_start(mask4f[:], mask4_d[:])
        mask4 = const.tile([4, P], f32r)
        nc.vector.tensor_copy(mask4[:], mask4f[:])  # f32 -> f32r rounding

        # ---- input DMAs.  k weights + image-B band 0 gate the first matmul;
        # q weights + image-A band 0 gate the q tile (and with it the first
        # exp); v weights later (v-branch runs as loop filler).
        wk_sb = sb.tile([P, 2 * 9 * P], bf16, tag="wk")
        pb_sb = sb.tile([P, 2 * 2500], bf16, tag="pb")
        wv_sb = sb.tile([P, 2 * 9 * P], bf16, tag="wv")
        wq_sb = sb.tile([P, 2 * 9 * P], bf16, tag="wq")
        pa_sb = sb.tile([P, 2 * 2500], bf16, tag="pa")
        bias_sb = sb.tile([P, 3], f32, tag="bias")

        def _img_band(dst, src, kc, r0, r1):
            nc.sync.dma_start(dst[:, kc * 2500 + r0 * 50:kc * 2500 + r1 * 50],
                              src[kc][:, r0 * 50:r1 * 50])

        for kc in range(2):
            nc.sync.dma_start(wk_sb[:, kc * 1152:(kc + 1) * 1152], wk[kc])
        for kc in range(2):
            _img_band(pb_sb, pad_b, kc, *DMA_BANDS[0])
        for kc in range(2):
            nc.sync.dma_start(wq_sb[:, kc * 1152:(kc + 1) * 1152], wq[kc])
        for kc in range(2):
            _img_band(pa_sb, pad_a, kc, *DMA_BANDS[0])
        for r in range(3):
            nc.sync.dma_start(bias_sb[:, r:r + 1], bias_d[r])
        for kc in range(2):
            nc.sync.dma_start(wv_sb[:, kc * 1152:(kc + 1) * 1152], wv[kc])
        for i, (r0, r1) in enumerate(DMA_BANDS):
            if i > 0:
                for kc in range(2):
                    _img_band(pb_sb, pad_b, kc, r0, r1)
                for kc in range(2):
                    _img_band(pa_sb, pad_a, kc, r0, r1)

        qT = sb.tile([P, T], bf16, tag="qT")
        qTf = sb.tile([P, T], f32, tag="qTf")   # fp32 copy for the residual
        kT = sb.tile([P, T], bf16, tag="kT")
        vT = sb.tile([P, T], bf16, tag="vT")
        # per t-chunk, per head: 33 cols = [v(32) | ones(1)]; the ones column
        # carries the softmax denominator through the AV matmul, and LDWEIGHTS
        # cost scales with weight columns (33 beats 64)
        vaug = sb.tile([P, N_TCH * 132], bf16, tag="vaug")
        nc.vector.memset(vaug[:], 0.0)
        ones_cols = vaug.rearrange("p (j h c) -> p j h c", h=4, c=33)[:, :, :, 32:33]
        nc.vector.memset(ones_cols, 1.0)
        outbuf = sb.tile([P, T], f32, tag="outbuf")

        # block-diagonal expanded q: qTx[32h:32h+32, h*256:(h+1)*256] = q strip
        # for the current l-tile, zeros elsewhere (ping-pong by l-tile parity)
        qTx0 = sb.tile([P, 4 * NL], bf16, tag="qTx0")
        qTx1 = sb.tile([P, 4 * NL], bf16, tag="qTx1")
        qTx = [qTx0, qTx1]
        nc.vector.memset(qTx[0][:], 0.0)
        nc.vector.memset(qTx[1][:], 0.0)

        # ---------------- emission helpers ----------------
        def branch_items(w_sb, img_sb, dest, role, dest2=None):
            """One branch as a list of closures: 18 matmuls + 1 drain per
            t-tile (so the scheduler can interleave at matmul granularity)."""
            items = []
            for (t0, nt, r0, nr) in T_TILES:
                cell = {}

                def mk_mm(t0, nt, r0, nr, kc, di, dj, mm, cell):
                    def run():
                        if mm == 0:
                            cell["ps"] = psumB.tile(
                                [P, nt], f32, tag="scratch", bufs=2,
                                name=f"br_{role}_{t0}")
                        pv = img_sb[:, kc * 2500:(kc + 1) * 2500].rearrange(
                            "p (r c) -> p r c", c=50)
                        w_ = w_sb[:, kc * 1152:(kc + 1) * 1152]
                        tap = di * 3 + dj
                        rhs = pv[:, r0 + di:r0 + di + nr, dj:dj + 48]
                        nc.tensor.matmul(cell["ps"][:],
                                         w_[:, tap * P:(tap + 1) * P], rhs,
                                         start=(mm == 0), stop=(mm == 17),
                                         skip_group_check=True)
                    return run

                mm = 0
                for kc in range(2):
                    for di in range(3):
                        for dj in range(3):
                            items.append(mk_mm(t0, nt, r0, nr, kc, di, dj, mm, cell))
                            mm += 1

                def mk_drain(t0, nt, cell):
                    def run():
                        nc.vector.tensor_scalar_add(
                            dest[:, t0:t0 + nt], cell["ps"][:],
                            bias_sb[:, role:role + 1])
                        if dest2 is not None:
                            nc.vector.tensor_scalar_add(
                                dest2[:, t0:t0 + nt], cell["ps"][:],
                                bias_sb[:, role:role + 1])
                    return run

                items.append(mk_drain(t0, nt, cell))
            return items

        def transpose_chunk(c):
            """vT chunk c -> vaug [t, d] blocks (+ ones col already set)."""
            tp = psumB.tile([P, P], bf16, tag="scratch", bufs=2, name=f"tp_{c}")
            nc.tensor.transpose(tp[:], vT[:, c * P:(c + 1) * P], identity[:])
            dst = vaug[:, c * 132:(c + 1) * 132].rearrange(
                "p (h c2) -> p h c2", c2=33)[:, :, 0:32]
            src = tp[:].rearrange("p (h c2) -> p h c2", c2=32)
            nc.vector.tensor_copy(dst, src)

        def build_qtx(li):
            dst = qTx[li % 2]
            for h in range(4):
                nc.vector.tensor_copy(
                    dst[32 * h:32 * h + 32, h * NL:(h + 1) * NL],
                    qT[32 * h:32 * h + 32, li * NL:(li + 1) * NL])

        sc_t, et_t, av_t = {}, {}, {}

        def scores(j):
            li, ch = divmod(j, N_TCH)
            t = psumB.tile([P, 4 * NL], f32, tag="sc", bufs=2, name=f"sc_{j}")
            sc_t[j] = t
            # fp32 PSUM matmul output is capped at one bank (N=512): two MMs
            # of 2 heads each (same stationary weights)
            for half in range(2):
                nc.tensor.matmul(t[:, half * 512:(half + 1) * 512],
                                 kT[:, ch * P:(ch + 1) * P],
                                 qTx[li % 2][:, half * 512:(half + 1) * 512],
                                 start=True, stop=True)

        def do_exp(j):
            et = ep.tile([P, 4 * NL], bf16, tag="et", bufs=ET_BUFS,
                         name=f"et_{j}")
            et_t[j] = et
            nc.scalar.activation(et[:], sc_t.pop(j)[:], AF.Exp, scale=-0.0625)

        def av(j):
            li, ch = divmod(j, N_TCH)
            if ch == 0:
                av_t[li] = psumB.tile([P, 2 * NL], f32, tag="avout", bufs=2,
                                      name=f"avout_{li}")
            outp = av_t[li]
            et = et_t.pop(j)
            for h in range(4):
                cp, pbase = NL * (h // 2), 64 * (h % 2)
                nc.tensor.matmul(
                    outp[pbase:pbase + 33, cp:cp + NL],
                    vaug[:, ch * 132 + 33 * h:ch * 132 + 33 * h + 33],
                    et[:, h * NL:(h + 1) * NL],
                    start=(ch == 0), stop=(ch == N_TCH - 1),
                    tile_position=(0, 64 * (h % 2)), skip_group_check=True)

        fin_state = {}

        def fin_step(li, sub):
            outp = av_t[li]
            l0 = li * NL
            if sub == 0:
                # DVE partition bases must be 32-aligned: stage the 4
                # denominator rows at partitions {0,32,64,96}, then compact to
                # partitions 0..3 with tiny SBUF->SBUF DMAs (DMAs are free-form)
                dstg = fin.tile([P, NL], f32, tag="dstg", bufs=2,
                                name=f"dstg_{li}")
                den4 = fin.tile([4, NL], f32, tag="den4", bufs=2,
                                name=f"den4_{li}")
                fin_state["den4"] = den4
                for h in range(4):
                    cp, pbase = NL * (h // 2), 64 * (h % 2)
                    nc.vector.tensor_copy(
                        dstg[32 * h:32 * h + 1, :],
                        outp[pbase + 32:pbase + 33, cp:cp + NL])
                for h in range(4):
                    nc.sync.dma_start(den4[h:h + 1, :],
                                      dstg[32 * h:32 * h + 1, :])
            elif sub == 1:
                den4r = fin.tile([4, NL], f32r, tag="den4r", bufs=2,
                                 name=f"den4r_{li}")
                fin_state["den4r"] = den4r
                with nc.allow_low_precision(reason="f32r recip: 2^-19 rel "
                                            "is ample for softmax denom"):
                    nc.vector.reciprocal(den4r[:], fin_state["den4"][:])
            elif sub == 2:
                bc_ps = psumB.tile([P, NL], f32, tag="scratch", bufs=2,
                                   name=f"bc_{li}")
                nc.tensor.matmul(bc_ps[:], mask4[:], fin_state["den4r"][:],
                                 start=True, stop=True)
                bc_sb = fin.tile([P, NL], f32, tag="bcsb", bufs=2,
                                 name=f"bcsb_{li}")
                fin_state["bc_sb"] = bc_sb
                nc.vector.tensor_copy(bc_sb[:], bc_ps[:])
            elif sub == 3:
                bc_sb = fin_state["bc_sb"]
                for h in range(4):
                    cp, pbase = NL * (h // 2), 64 * (h % 2)
                    nc.vector.tensor_tensor(
                        outbuf[h * 32:(h + 1) * 32, l0:l0 + NL],
                        outp[pbase:pbase + 32, cp:cp + NL],
                        bc_sb[32 * h:32 * h + 32, :], op=OP.mult)
            elif sub == 4:
                for h in range(4):
                    nc.vector.tensor_tensor(
                        outbuf[h * 32:(h + 1) * 32, l0:l0 + NL],
                        outbuf[h * 32:(h + 1) * 32, l0:l0 + NL],
                        qTf[h * 32:(h + 1) * 32, l0:l0 + NL], op=OP.add)
            elif sub == 5:
                nc.sync.dma_start(out_d[:, l0:l0 + NL], outbuf[:, l0:l0 + NL])
                del av_t[li]

        # ---------------- upfront: k-branch || l-tile-0 attention ----------
        k_items = branch_items(wk_sb, pb_sb, kT, 1)
        q_items = branch_items(wq_sb, pa_sb, qT, 0, dest2=qTf)
        v_items = branch_items(wv_sb, pb_sb, vT, 2)

        KB = 19  # items per branch t-tile (18 matmuls + drain)
        for it in k_items[0:KB]:          # k t-tile 0
            it()
        for it in q_items[0:KB]:          # q t-tile 0 (covers l-tiles 0..1)
            it()
        build_qtx(0)
        emitted_exp = -1
        for ti in range(5):               # k t-tiles 1-4 alternate with the
            if ti > 0:                    # l0 scores/exps they unlock
                for it in k_items[ti * KB:(ti + 1) * KB]:
                    it()
            for ch in range(*TILE_CHUNKS[ti]):
                scores(ch)
                do_exp(ch)
                emitted_exp = ch
        for it in q_items[KB:2 * KB]:     # q t-tile 1 (l-tiles 2..3)
            it()
        build_qtx(1)
        scores(18)

        # ---------------- main loop: one step per exp ----------------------
        # fillers: v-branch (with transposes after each tile drain), then
        # q t-tiles 2-4.  AV is emission-gated on vaug availability.
        fillers = []
        v_done_chunks = [0]
        for ti in range(5):
            fillers.extend(v_items[ti * KB:(ti + 1) * KB])

            def mk_tp(ti):
                def run():
                    for c in range(*TILE_CHUNKS[ti]):
                        transpose_chunk(c)
                    v_done_chunks[0] = TILE_CHUNKS[ti][1]
                return run

            fillers.append(mk_tp(ti))
        for ti in range(2, 5):
            fillers.extend(q_items[ti * KB:(ti + 1) * KB])

        fin_queue = []   # (li, sub) pending finalize substeps
        av_next = [0]

        def try_avs(j, budget):
            n = 0
            while av_next[0] <= j - 2 and n < budget:
                jj = av_next[0]
                li_a, ch_a = divmod(jj, N_TCH)
                if ch_a >= v_done_chunks[0]:
                    break  # its vaug chunk is not emitted yet
                if ch_a == 0 and li_a >= 2:
                    # avout bufs=2: force finalize(li_a-2) fully emitted first
                    while fin_queue and fin_queue[0][0] <= li_a - 2:
                        fin_step(*fin_queue.pop(0))
                av(jj)
                if ch_a == N_TCH - 1:
                    fin_queue.extend((li_a, s) for s in range(6))
                av_next[0] += 1
                n += 1

        for j in range(N_TCH, NJ):
            li, jl = divmod(j, N_TCH)
            # PE fillers: 5/step while the v-branch is pending, 2 after
            nfill = 5 if v_done_chunks[0] < N_TCH else 2
            for _ in range(nfill):
                if fillers:
                    fillers.pop(0)()
            if jl == 15 and li + 1 < N_LT:
                build_qtx(li + 1)
            if j + 1 < NJ:
                scores(j + 1)
            do_exp(j)
            # keep AV within the et ring (lag < ET_BUFS-1), else force-drain
            while av_next[0] < j - (ET_BUFS - 3):
                if av_next[0] % N_TCH >= v_done_chunks[0]:
                    while fillers and av_next[0] % N_TCH >= v_done_chunks[0]:
                        fillers.pop(0)()
                try_avs(j, 1)
            try_avs(j, 2)
            if fin_queue:
                fin_step(*fin_queue.pop(0))

        # ---------------- tail ----------------
        while fillers:
            fillers.pop(0)()
        while av_next[0] < NJ:
            try_avs(NJ + 1, 4)
        while fin_queue:
            fin_step(*fin_queue.pop(0))

    nc.compile()
    return nc


def _fold_weights(dw_w, bn_gamma, bn_beta, bn_mean, bn_var, pw_w, pw_b, lin_w):
    """Fold BN + pointwise conv + linear (+ depthwise taps) per branch.

    Returns Wtap [6, 9, 256, 256] (float32) and bias c [6, 256]."""
    dw = dw_w.astype(np.float64)
    g = bn_gamma.astype(np.float64)
    b = bn_beta.astype(np.float64)
    m = bn_mean.astype(np.float64)
    v = bn_var.astype(np.float64)
    pw = pw_w.astype(np.float64)
    pb = pw_b.astype(np.float64)
    lw = lin_w.astype(np.float64)

    scale = g / np.sqrt(v + EPS)                      # [6, 256]
    shift = b - m * scale                             # [6, 256]
    M = np.einsum("noc,ncd->nod", lw, pw)             # lin @ pw  [6, 256, 256]
    W = M * scale[:, None, :]                         # [6, 256(o), 256(c)]
    c = np.einsum("noc,nc->no", M, shift) + np.einsum("noc,nc->no", lw, pb)
    # taps: Wtap[n, di*3+dj, o, c] = W[n, o, c] * dw[n, c, di, dj]
    Wtap = W[:, None, :, :] * dw.transpose(0, 2, 3, 1).reshape(6, 9, 1, 256)
    return Wtap.astype(np.float32), c.astype(np.float32)


def _bf16(a):
    import ml_dtypes
    return a.astype(ml_dtypes.bfloat16)


def _pad_images(x):
    """x [B, T, 256] -> per batch channel-major zero-padded bf16 [2,128,2500]."""
    out = np.zeros((B, 2, P, 50, 50), dtype=np.float32)
    img = np.ascontiguousarray(x.transpose(0, 2, 1)).reshape(B, DIM, HW, HW)
    out[:, :, :, 1:49, 1:49] = img.reshape(B, 2, P, HW, HW)
    return _bf16(out.reshape(B, 2, P, 2500))


def _wtap_lhsT(Wtap, branch, g):
    """Pack lhsT layout [2, 128, 9*128] for a branch restricted to quad g."""
    rows = slice(g * P, (g + 1) * P)
    out = np.empty((2, P, 9 * P), dtype=np.float32)
    for kc in range(2):
        for tap in range(9):
            blk = Wtap[branch, tap][rows, kc * P:(kc + 1) * P]  # [128 o, 128 c]
            out[kc, :, tap * P:(tap + 1) * P] = blk.T
    return _bf16(out)


def kernel(x1, x2, dw_w, bn_gamma, bn_beta, bn_mean, bn_var, pw_w, pw_b, lin_w,
           h1=HW, w1=HW, h2=HW, w2=HW):
    global _PROGRAM
    from concourse.bass_utils import run_bass_kernel_spmd

    x1 = np.asarray(x1, dtype=np.float32)
    x2 = np.asarray(x2, dtype=np.float32)

    Wtap, c = _fold_weights(np.asarray(dw_w), np.asarray(bn_gamma),
                            np.asarray(bn_beta), np.asarray(bn_mean),
                            np.asarray(bn_var), np.asarray(pw_w),
                            np.asarray(pw_b), np.asarray(lin_w))
    pad1 = _pad_images(x1)   # [B, 2, 128, 2500]
    pad2 = _pad_images(x2)
    mask4 = np.zeros((4, P), dtype=np.float32)
    for h in range(4):
        mask4[h, 32 * h:32 * h + 32] = 1.0

    if _PROGRAM is None:
        _PROGRAM = _build_program()
    nc = _PROGRAM

    # core layout: core = m*4 + b*2 + g
    # map m=0: o1 = att(q=br0(x1), k=br4(x2), v=br5(x2)) + q1
    # map m=1: o2 = att(q=br3(x2), k=br1(x1), v=br2(x1)) + q2
    in_maps = []
    for m in range(2):
        qbr, kbr, vbr = (0, 4, 5) if m == 0 else (3, 1, 2)
        pa, pb_ = (pad1, pad2) if m == 0 else (pad2, pad1)
        for b in range(2):
            for g in range(2):
                bias = np.stack([c[qbr, g * P:(g + 1) * P],
                                 c[kbr, g * P:(g + 1) * P],
                                 c[vbr, g * P:(g + 1) * P]])[:, :, None]
                in_maps.append({
                    "pad_a": np.ascontiguousarray(pa[b]),
                    "pad_b": np.ascontiguousarray(pb_[b]),
                    "wq": _wtap_lhsT(Wtap, qbr, g),
                    "wk": _wtap_lhsT(Wtap, kbr, g),
                    "wv": _wtap_lhsT(Wtap, vbr, g),
                    "bias": np.ascontiguousarray(bias),
                    "mask4": mask4,
                })

    global _last_in_maps
    _last_in_maps = in_maps
    res = run_bass_kernel_spmd(nc, in_maps, list(range(N_CORES)))

    o = np.empty((2, 2, HEADS, T, DH), dtype=np.float32)
    for m in range(2):
        for b in range(2):
            for g in range(2):
                core = m * 4 + b * 2 + g
                blk = res.results[core]["out"].reshape(4, DH, T)
                o[m, b, 4 * g:4 * g + 4] = blk.transpose(0, 2, 1)
    o1 = o[0].reshape(B, T, HEADS * DH)
    o2 = o[1].reshape(B, T, HEADS * DH)
    return o1, o2
